# revision 121
# baseline (speedup 1.0000x reference)
"""BBox window attention kernel for 8 TRN2 NeuronCores.

Sharding: data-parallel over batch B=8 -> one batch element per core.
Each core computes the full attention for its batch element; no collectives.

Per-core pipeline:
  phase A (chunk-pipelined): x loads via gpsimd casting DMA (f32->bf16 in
  flight; chunk 0 rides the SP HWDGE queue as f32 and transposes directly),
  PE-transpose to xT kept as an fp8 hi/lo residual pair, then per 512-token
  chunk emit the covered qk projections, the s0 global-token scores, the v
  projection, and the global-token output accumulation
  o0T[e,h] += v_tile^T @ P0T_tile.  The qk/v projections run as fp8e4m3
  DoubleRow matmuls (half the bf16 PE cycles) with a 3-term residual
  correction that matches bf16 accuracy; projection blocks lag the x
  transposes so they never wait on the (later-arriving) weight DMAs.
  phase B: windowed attention (S matmuls split by head-half across two PSUM
  banks in bf16, batched exp without max-subtraction, DVE sum/recip, GpSimd
  broadcast-normalize, PE-transpose P, V^T @ P^T), with drains spread
  across DVE/ACT, output-projection tiles trickled two per win_back with a
  one-iteration arming delay, and the first supergroup's fronts emitted
  during the phase-A tail to hide the softmax-normalizer chain.
"""

import sys

for _p in ("/opt/trn_rl_repo",):
    if _p not in sys.path:
        sys.path.insert(0, _p)

import numpy as np

import concourse.bass as bass
import concourse.tile as tile
from concourse import bacc, mybir
from concourse.bass_utils import run_bass_kernel_spmd
from concourse.masks import make_identity

F32 = mybir.dt.float32
BF16 = mybir.dt.bfloat16
FP8 = mybir.dt.float8e4

# The qk/v projections run as fp8e4m3 DoubleRow matmuls (half the PE cycles
# of bf16) with a residual correction: x ~ Xh + Xl and W ~ Wh + Wl where
# W = WSCALE*w keeps the weights in fp8's normal range.  The three retained
# product terms (Xh@Wh + Xl@Wh + Xh@Wl) land in one PSUM accumulation and
# reach bf16-level accuracy; the WSCALE factor rides through qT/kT/v and is
# removed by the exp scale (1/WSCALE^2) and the output drains (1/WSCALE).
WSCALE = 32.0

B, T_FULL, D = 8, 4097, 512
H, WIN, d_head = 8, 64, 64
N_CORES = 8
CH = 4          # head-pair chunks (128 features each)
KC = 4          # contraction chunks of 128 over D
TBS = 456       # token block size for feature-major projections (<=512 psum bank)
SCALE = float(d_head) ** -0.5


def _emit(nc, tc, x_d, wqkv_d, wout_d, out_d, T):
    TW = T - 1                 # window tokens
    NW = TW // WIN             # number of windows
    WGN = NW // 8              # window groups (8 windows each)
    assert NW % 8 == 0
    TQ = (T + 127) // 128      # token tiles of 128
    NTB = (T + TBS - 1) // TBS  # projection token blocks
    VT = TW // 128             # v tiles (tokens 1..TW)
    assert TW % 128 == 0
    NXC = (TQ - 1) // 4        # full x chunks of 4 tiles (512 tokens)
    assert NXC * 4 == TQ - 1 and T == NXC * 512 + 1

    def pool(name, **kw):
        return tc.tile_pool(name=name, **kw)

    with pool("persist", bufs=1) as persist, \
         pool("stats", bufs=4) as stats, \
         pool("pp", bufs=4) as pp, \
         pool("psum_r0", bufs=5, space="PSUM") as pbig, \
         pool("psum_r64", bufs=2, space="PSUM") as pr64:

        # PSUM discipline (hardware-validated): all matmul groups landing in
        # one physical bank must share the same tile_position ROW (= lhsT/rhs
        # partition base).  pbig tiles and the o0 bank only ever host row-0
        # groups; pr64's "r64" banks host row-64 groups (odd head-half S
        # tiles / odd window-parity O tiles).
        ident = persist.tile([128, 128], BF16)
        make_identity(nc, ident)
        ident128_32 = persist.tile([128, 128], F32)
        make_identity(nc, ident128_32)

        wq_hi = persist.tile([128, KC, 3 * D], FP8)
        wq_lo = persist.tile([128, KC, 3 * D], FP8)
        wout_sb = persist.tile([128, KC, D], BF16)
        qT = persist.tile([128, CH, T], BF16)
        kT = persist.tile([128, CH, T], BF16)
        v_sb = persist.tile([128, VT, D], BF16)
        v0_sb = persist.tile([1, D], BF16)
        q0all = persist.tile([128, CH, 8], BF16)
        P0_sb = persist.tile([8, T], BF16)
        P0T_sb = persist.tile([128, VT, 8], BF16)
        p00_sb = persist.tile([1, 8], BF16)
        s0part = persist.tile([8, NTB], F32)
        s0stat = persist.tile([8, 4], F32)  # cols: -, -, sum, recip
        ident32 = persist.tile([8, 8], F32)
        r0row = persist.tile([1, 8], F32)
        r0bc = persist.tile([128, 8], F32)
        # o0 accumulates in four independent slots of one PSUM bank; PSUM
        # zero-regions are bank-granular, so a start=True on one slot would
        # clobber the others' partials.  Memset once and accumulate without
        # start instead.
        o0_ps = pr64.tile([128, CH, 8], F32, tag="o0", bufs=1)
        nc.vector.memset(o0_ps[:, :, :], 0.0)

        # Window wj (0..15 within a 16-window supergroup) maps to bits
        # (u, b1, s2) = (wj&1, (wj>>1)&1, wj>>2 in 0..3).  Layouts keep
        # every matmul's lhsT/rhs partition base equal and the
        # tile_position row fixed per PSUM tile (hardware requirement):
        #   S tile (per head-half r):  [64*b1 + q, slot=2*s2+u, k]
        #   PT (transposed P):         [64*u + k, slab=4*r+s2, 64*b1 + q]
        #   O tile (per parity u):     [64*r + e, slot=2*s2+b1, q]
        WG2 = WGN // 2  # supergroups of 16 windows

        def win_front(wg2, c):
            """S matmuls + softmax for one iteration; returns P tiles."""
            P_sb = [None, None]
            for r in range(2):
                sp = (pbig if r == 0 else pr64).tile(
                    [128, 8, WIN], F32, tag=("big" if r == 0 else "r64"))
                for wj in range(16):
                    u, b1, s2 = wj & 1, (wj >> 1) & 1, wj >> 2
                    col0 = 1 + WIN * (16 * wg2 + wj)
                    nc.tensor.matmul(
                        sp[64 * b1:64 * b1 + 64, 2 * s2 + u, :],
                        qT[64 * r:64 * r + 64, c, col0:col0 + WIN],
                        kT[64 * r:64 * r + 64, c, col0:col0 + WIN],
                        start=True,
                        stop=True,
                    )
                pb = pp.tile([128, 8, WIN], BF16, tag="P", bufs=6)
                P_sb[r] = pb
                nc.scalar.activation(
                    pb[:, :, :].rearrange("p a b -> p (a b)"),
                    sp[:, :, :].rearrange("p a b -> p (a b)"),
                    mybir.ActivationFunctionType.Exp,
                    bias=0.0, scale=SCALE / (WSCALE * WSCALE),
                )
                sums = stats.tile([128, 8, 1], F32, tag="sums")
                nc.vector.reduce_sum(
                    sums[:, :, :], pb[:, :, :], axis=mybir.AxisListType.X,
                    op=mybir.AluOpType.add,
                )
                rs = stats.tile([128, 8, 1], F32, tag="rs")
                nc.vector.reciprocal(rs[:, :, :], sums[:, :, :])
                nc.gpsimd.tensor_tensor(
                    pb[:, :, :], pb[:, :, :],
                    rs[:, :, :].broadcast_to([128, 8, WIN]),
                    op=mybir.AluOpType.mult,
                )
            return P_sb

        # round-robin PSUM->SBUF drains across DVE / ACT (GPSIMD cannot read
        # PSUM on hardware); optional scale folds a rescale into the drain
        _rr = [0]

        def rr_drain(dst, src, scale=None):
            e = _rr[0] % 2
            _rr[0] += 1
            if scale is None:
                if e == 0:
                    nc.vector.tensor_copy(dst, src)
                else:
                    nc.scalar.copy(dst, src)
            else:
                if e == 0:
                    nc.vector.tensor_scalar_mul(dst, src, scale)
                else:
                    nc.scalar.activation(
                        dst, src, mybir.ActivationFunctionType.Identity,
                        bias=0.0, scale=scale,
                    )

        # ---- phase A: chunk-pipelined load + transpose + projections ----
        with pool("xstage", bufs=3) as xstage, pool("xT", bufs=1) as xTpool:
            # One tile per DoubleRow kc-pair, token t at column t.  The
            # Ldweights ISA caps the kc-pair stride at 4096, so the main
            # tiles hold tokens 0..4095 and a small aux pair holds tokens
            # 3648..4096 for the final projection block and v tile.
            XAB = T - 1 - TBS * (NTB - 1)  # aux covers this many + 1 tokens
            xT_hi = [xTpool.tile([128, 2, 4096], FP8, name=f"xT_hi{i}",
                                 tag=f"xh{i}") for i in range(2)]
            xT_lo = [xTpool.tile([128, 2, 4096], FP8, name=f"xT_lo{i}",
                                 tag=f"xl{i}") for i in range(2)]
            xA_hi = [xTpool.tile([128, 2, 512], FP8, name=f"xA_hi{i}",
                                 tag=f"xah{i}") for i in range(2)]
            xA_lo = [xTpool.tile([128, 2, 512], FP8, name=f"xA_lo{i}",
                                 tag=f"xal{i}") for i in range(2)]
            # chunk 0 also keeps a bf16 transpose: its projections run on the
            # bf16 path because the fp8 weight split isn't ready that early
            xT0 = xstage.tile([128, KC, 464], BF16, tag="xT0", bufs=1)
            wqkv_st = xstage.tile([128, KC, 3 * D], BF16, tag="wst", bufs=1)

            def x_chunk_dma(c):
                r00 = 512 * c
                if c == 0:
                    # chunk 0: tiles 0-1 ride the SP HWDGE queue as f32
                    # (transposed directly, no cast) so the first transpose
                    # starts ~2.5us in; tiles 2-3 stream as a bf16 casting
                    # DMA ahead of the weights on SWDGE
                    xb = []
                    for j in range(2):
                        xs0 = xstage.tile([128, 512], F32, tag="xs0", bufs=3)
                        nc.sync.dma_start(
                            out=xs0[:, :],
                            in_=x_d[r00 + 128 * j:r00 + 128 * (j + 1), :],
                        )
                        xb.append(xs0)
                    x23 = xstage.tile([128, 2, 512], BF16, tag="xb")
                    nc.gpsimd.dma_start(
                        out=x23[:, :, :],
                        in_=x_d[r00 + 256:r00 + 512, :].rearrange(
                            "(j p) e -> p j e", p=128),
                    )
                    xb.append(x23)
                    return xb
                if c == 1:
                    # half on SP+cast (arrives early), half on SWDGE
                    xb = xstage.tile([128, 4, 512], BF16, tag="xb")
                    for j in range(2):
                        xs0 = xstage.tile([128, 512], F32, tag="xs0", bufs=3)
                        nc.sync.dma_start(
                            out=xs0[:, :],
                            in_=x_d[r00 + 128 * j:r00 + 128 * (j + 1), :],
                        )
                        if j % 2 == 0:
                            nc.vector.tensor_copy(xb[:, j, :], xs0[:, :])
                        else:
                            nc.scalar.copy(xb[:, j, :], xs0[:, :])
                    nc.gpsimd.dma_start(
                        out=xb[:, 2:4, :],
                        in_=x_d[r00 + 256:r00 + 512, :].rearrange(
                            "(j p) e -> p j e", p=128),
                    )
                    return xb
                xb = xstage.tile([128, 4, 512], BF16, tag="xb")
                nc.gpsimd.dma_start(
                    out=xb[:, :, :],
                    in_=x_d[r00:r00 + 512, :].rearrange(
                        "(j p) e -> p j e", p=128),
                )
                return xb

            def x_chunk_transpose(xb, c):
                for j in range(4):
                    r0 = 512 * c + 128 * j
                    if c == 0 and j < 2:  # f32 straight from the SP loads
                        tp = pbig.tile([128, KC, 128], F32, tag="big")
                        for kc in range(KC):
                            nc.tensor.transpose(
                                tp[:, kc, :],
                                xb[j][:, 128 * kc:128 * (kc + 1)],
                                ident128_32[:, :],
                            )
                    elif c == 0:  # bf16 from the SWDGE casting DMA
                        tp = pbig.tile([128, KC, 128], BF16, tag="big")
                        for kc in range(KC):
                            nc.tensor.transpose(
                                tp[:, kc, :],
                                xb[2][:, j - 2, 128 * kc:128 * (kc + 1)],
                                ident[:, :],
                            )
                    else:
                        tp = pbig.tile([128, KC, 128], BF16, tag="big")
                        for kc in range(KC):
                            nc.tensor.transpose(
                                tp[:, kc, :],
                                xb[:, j, 128 * kc:128 * (kc + 1)],
                                ident[:, :],
                            )
                    if c == 0 and r0 < 464:
                        # first: the bf16 chunk-0 copy that the early
                        # projections wait on
                        wd = min(128, 464 - r0)
                        if j % 2 == 0:
                            nc.vector.tensor_copy(xT0[:, :, r0:r0 + wd],
                                                  tp[:, :, 0:wd])
                        else:
                            nc.scalar.copy(xT0[:, :, r0:r0 + wd],
                                           tp[:, :, 0:wd])
                    for k2 in range(2):
                        hi = xT_hi[k2][:, :, r0:r0 + 128]
                        lo = xT_lo[k2][:, :, r0:r0 + 128]
                        tpk = tp[:, 2 * k2:2 * k2 + 2, :]
                        nc.scalar.copy(hi, tpk)
                        nc.vector.scalar_tensor_tensor(
                            lo, tpk, 1.0, hi,
                            op0=mybir.AluOpType.mult,
                            op1=mybir.AluOpType.subtract,
                        )

            def qkT_block(tb):
                c0 = TBS * tb
                w = min(TBS, T - c0)
                for jb in range(8):
                    ps = pbig.tile([128, TBS], F32, tag="big")
                    if tb == 0:
                        for kc in range(KC):
                            nc.tensor.matmul(
                                ps[:, :w],
                                wqkv_st[:, kc, 128 * jb:128 * (jb + 1)],
                                xT0[:, kc, c0:c0 + w],
                                start=(kc == 0),
                                stop=(kc == KC - 1),
                            )
                    else:
                        if tb == NTB - 1:  # tail block reads the aux tiles
                            xh, xl, cc = xA_hi, xA_lo, 0
                        else:
                            xh, xl, cc = xT_hi, xT_lo, c0
                        terms = [(wq_hi, xh), (wq_hi, xl), (wq_lo, xh)]
                        for ti, (wt, xt) in enumerate(terms):
                            for k2 in range(2):
                                nc.tensor.matmul(
                                    ps[:, :w],
                                    wt[:, 2 * k2:2 * k2 + 2,
                                       128 * jb:128 * (jb + 1)],
                                    xt[k2][:, :, cc:cc + w],
                                    start=(ti == 0 and k2 == 0),
                                    stop=(ti == 2 and k2 == 1),
                                    perf_mode=mybir.MatmulPerfMode.DoubleRow,
                                )
                    if jb < 4:
                        dst = qT[:, jb, c0:c0 + w]
                    else:
                        dst = kT[:, jb - 4, c0:c0 + w]
                    rr_drain(dst, ps[:, :w],
                             scale=(WSCALE if tb == 0 else None))
                if tb == 0:
                    # q0all column h holds q0 of head h only in head h's
                    # partition range of its chunk and zeros elsewhere, so the
                    # four chunk matmuls of s0 accumulate cleanly.
                    nc.vector.memset(q0all[:, :, :], 0.0)
                    for h in range(H):
                        r0 = 64 * (h % 2)
                        nc.vector.tensor_copy(
                            q0all[r0:r0 + 64, h // 2, h:h + 1],
                            qT[r0:r0 + 64, h // 2, 0:1],
                        )
                # global-token scores for this block; exp without the
                # max-subtraction stabilizer is safe for these magnitudes
                ps0 = pbig.tile([8, TBS], F32, tag="big")
                for c in range(CH):
                    nc.tensor.matmul(
                        ps0[:, :w],
                        q0all[:, c, :],
                        kT[:, c, c0:c0 + w],
                        start=(c == 0),
                        stop=(c == CH - 1),
                    )
                nc.scalar.activation(
                    P0_sb[:, c0:c0 + w], ps0[:, :w],
                    mybir.ActivationFunctionType.Exp,
                    bias=0.0, scale=SCALE / (WSCALE * WSCALE),
                    accum_out=s0part[:, tb:tb + 1],
                )
                if tb == 0:
                    # token 0's v row and P0 column only need block 0; doing
                    # them here keeps them off the phase-A tail critical path
                    psv0 = pbig.tile([1, D], F32, tag="big")
                    for kc in range(KC):
                        nc.tensor.matmul(
                            psv0[:, :],
                            xT0[:, kc, 0:1],
                            wqkv_st[:, kc, 2 * D:3 * D],
                            start=(kc == 0),
                            stop=(kc == KC - 1),
                        )
                    nc.vector.tensor_scalar_mul(v0_sb[:, :], psv0[:, :],
                                                WSCALE)
                    tp8 = pbig.tile([1, 8], BF16, tag="big")
                    nc.tensor.transpose(tp8[0:1, :], P0_sb[:, 0:1],
                                        ident[0:8, 0:8])
                    nc.vector.tensor_copy(p00_sb[:, :], tp8[0:1, :])

            def v_tile(vt):
                c0 = 1 + 128 * vt
                ps = pbig.tile([128, D], F32, tag="big")
                if vt < 3:  # fully inside chunk 0: bf16 path
                    for kc in range(KC):
                        nc.tensor.matmul(
                            ps[:, :],
                            xT0[:, kc, c0:c0 + 128],
                            wqkv_st[:, kc, 2 * D:3 * D],
                            start=(kc == 0),
                            stop=(kc == KC - 1),
                        )
                else:
                    if vt == VT - 1:  # tokens 3969..4096 live in the aux pair
                        xh, xl, cc = xA_hi, xA_lo, c0 - TBS * (NTB - 1)
                    else:
                        xh, xl, cc = xT_hi, xT_lo, c0
                    terms = [(xh, wq_hi), (xl, wq_hi), (xh, wq_lo)]
                    for ti, (xt, wt) in enumerate(terms):
                        for k2 in range(2):
                            nc.tensor.matmul(
                                ps[:, :],
                                xt[k2][:, :, cc:cc + 128],
                                wt[:, 2 * k2:2 * k2 + 2, 2 * D:3 * D],
                                start=(ti == 0 and k2 == 0),
                                stop=(ti == 2 and k2 == 1),
                                perf_mode=mybir.MatmulPerfMode.DoubleRow,
                            )
                rr_drain(v_sb[:, vt, :], ps[:, :],
                         scale=(WSCALE if vt < 3 else None))

            def p0t_o0_tile(vt):
                c0 = 1 + 128 * vt
                tp8 = pbig.tile([128, 8], BF16, tag="big")
                nc.tensor.transpose(tp8[:, :], P0_sb[:, c0:c0 + 128],
                                    ident[0:8, 0:8])
                nc.vector.tensor_copy(P0T_sb[:, vt, :], tp8[:, :])
                for c4 in range(CH):
                    nc.tensor.matmul(
                        o0_ps[:, c4, :],
                        v_sb[:, vt, 128 * c4:128 * (c4 + 1)],
                        P0T_sb[:, vt, :],
                        start=False,
                        stop=False,
                        skip_group_check=True,
                    )

            # DMA order (the DMA engine pool is a serial ~360B/ns resource,
            # so arrival order == emission order): chunk 0 (SP queue), the
            # W_v columns (1MB, unblocks the early v tiles), chunk 1, then
            # the W_qk columns (2MB, only needed once qkT starts two chunks
            # later), the remaining chunks, the tail token, wout.
            xbs = [x_chunk_dma(0), x_chunk_dma(1)]
            for kcp in range(2):
                nc.gpsimd.dma_start(
                    out=wqkv_st[:, 2 * kcp:2 * kcp + 2, 2 * D:3 * D],
                    in_=wqkv_d[256 * kcp:256 * (kcp + 1), 2 * D:3 * D]
                    .rearrange("(kc p) e -> p kc e", p=128),
                )
            xbs.append(x_chunk_dma(2))
            xbs.append(x_chunk_dma(3))
            for kcp in range(2):
                nc.gpsimd.dma_start(
                    out=wqkv_st[:, 2 * kcp:2 * kcp + 2, 0:2 * D],
                    in_=wqkv_d[256 * kcp:256 * (kcp + 1), 0:2 * D]
                    .rearrange("(kc p) e -> p kc e", p=128),
                )
            for c in range(4, NXC):
                xbs.append(x_chunk_dma(c))
            xbt = xstage.tile([1, 512], BF16, tag="xbt", bufs=1)
            nc.gpsimd.dma_start(out=xbt[:, :], in_=x_d[T - 1:T, :])
            nc.gpsimd.dma_start(
                out=wout_sb[:, :, :],
                in_=wout_d[:, :].rearrange("(kc p) e -> p kc e", p=128),
            )

            def weight_prep(col_lo, col_hi):
                # W = WSCALE*w -> fp8 hi, lo = W - hi (fp8, partly denormal:
                # the residual term only needs a few percent accuracy)
                for kcp in range(2):
                    sl = slice(col_lo, col_hi)
                    wsrc = wqkv_st[:, 2 * kcp:2 * kcp + 2, sl]
                    hi = wq_hi[:, 2 * kcp:2 * kcp + 2, sl]
                    if kcp % 2 == 0:
                        nc.vector.tensor_scalar_mul(hi, wsrc, WSCALE)
                    else:
                        nc.scalar.activation(
                            hi, wsrc,
                            mybir.ActivationFunctionType.Identity,
                            bias=0.0, scale=WSCALE,
                        )
                    nc.vector.scalar_tensor_tensor(
                        wq_lo[:, 2 * kcp:2 * kcp + 2, sl], wsrc, WSCALE,
                        hi, op0=mybir.AluOpType.mult,
                        op1=mybir.AluOpType.subtract,
                    )

            # Chunk-pipelined emission.  qkT blocks lag the transposes by two
            # chunks so PE never waits for the (later-arriving) W_qk columns;
            # the first block runs on the bf16 path with the raw staged
            # weights, later blocks on fp8 (prep emitted once the bf16 block
            # is in flight).
            v_done = 0
            o0_done = 0
            qk_done = 0

            def emit_qk(qk_target):
                nonlocal qk_done
                while qk_done < qk_target:
                    qkT_block(qk_done)
                    qk_done += 1
                    if qk_done == 1:
                        weight_prep(0, 2 * D)  # fp8 split for the qk columns

            def emit_v_o0(c):
                nonlocal v_done, o0_done
                # one-tile margin so a v tile never chases the drain of an
                # x tile transposed in the same iteration
                tokens = 512 * (c + 1) - (128 if c + 1 < NXC else 0)
                while v_done < VT and 129 + 128 * v_done <= tokens:
                    v_tile(v_done)
                    v_done += 1
                p0cols = TBS * qk_done
                while o0_done < v_done and 129 + 128 * o0_done <= p0cols:
                    p0t_o0_tile(o0_done)
                    o0_done += 1

            for c in range(NXC):
                emit_qk(max(0, c - 2))  # uses data >= 3 chunks old
                x_chunk_transpose(xbs[c], c)
                if c == 0:
                    weight_prep(2 * D, 3 * D)  # fp8 split for the v columns
                emit_v_o0(c)

            # aux pair for the final block: tokens 3648..4095 copied from the
            # main tiles on the (otherwise idle) Pool engine
            for k2 in range(2):
                nc.gpsimd.tensor_copy(
                    xA_hi[k2][:, :, 0:XAB],
                    xT_hi[k2][:, :, TBS * (NTB - 1):4096])
                nc.gpsimd.tensor_copy(
                    xA_lo[k2][:, :, 0:XAB],
                    xT_lo[k2][:, :, TBS * (NTB - 1):4096])

            # catch up the lagged projection blocks, then the tail token;
            # the first window fronts interleave here (they only need the
            # first supergroup's qT/kT columns, done long ago) so their
            # softmax chains run during the remaining projection work
            warm = []
            while qk_done < NTB - 1:
                qkT_block(qk_done)
                qk_done += 1
                if len(warm) < 2:
                    warm.append(win_front(0, len(warm)))

            # tail token T-1: transpose into xT column T-1
            tp = pbig.tile([128, KC, 2], BF16, tag="big")
            for kc in range(KC):
                nc.tensor.transpose(
                    tp[:, kc, 0:1],
                    xbt[0:1, 128 * kc:128 * (kc + 1)],
                    ident[0:1, 0:1],
                )
            for k2 in range(2):
                nc.vector.tensor_copy(xA_hi[k2][:, :, XAB:XAB + 1],
                                      tp[:, 2 * k2:2 * k2 + 2, 0:1])
                nc.vector.scalar_tensor_tensor(
                    xA_lo[k2][:, :, XAB:XAB + 1],
                    tp[:, 2 * k2:2 * k2 + 2, 0:1],
                    1.0, xA_hi[k2][:, :, XAB:XAB + 1],
                    op0=mybir.AluOpType.mult, op1=mybir.AluOpType.subtract,
                )

            qkT_block(NTB - 1)
            while v_done < VT:
                v_tile(v_done)
                v_done += 1
            while o0_done < VT:
                p0t_o0_tile(o0_done)
                o0_done += 1

            # warm the window pipeline: the first two supergroup fronts are
            # emitted here, sandwiching the phase-A tail (o0 close + the s0
            # normalizer chain) so PE keeps streaming while ACT/DVE finish
            # the tb=8 exp -> reduce -> recip chain.
            for c4 in range(CH):
                nc.tensor.matmul(
                    o0_ps[:, c4, :],
                    v0_sb[:, 128 * c4:128 * (c4 + 1)],
                    p00_sb[:, :],
                    start=False,
                    stop=True,
                    skip_group_check=True,
                )
            nc.vector.reduce_sum(
                s0stat[:, 2:3], s0part[:, :], axis=mybir.AxisListType.X,
                op=mybir.AluOpType.add,
            )
            nc.vector.reciprocal(s0stat[:, 3:4], s0stat[:, 2:3])
            make_identity(nc, ident32)
            r0p = pbig.tile([1, 8], F32, tag="big")
            nc.tensor.transpose(r0p[0:1, :], s0stat[:, 3:4], ident32[:, :])
            # attnT column 0 keeps the same WSCALE factor as the window
            # columns; the output-projection drain removes it for all alike
            nc.vector.tensor_copy(r0row[:, :], r0p[0:1, :])
            nc.gpsimd.partition_broadcast(r0bc[:, :], r0row[0:1, :])
            warm.append(win_front(0, 2))


        # ---- windowed attention + output projection ----
        with pool("attnT", bufs=1) as apool, \
             pool("osb2", bufs=8) as posb2:
            # second output staging pool, carved from the SBUF the xT pool
            # released: together with osb it gives 9 rotation slots so the
            # final output flush never waits on store-DMA completion
            attnT = apool.tile([128, CH, T], BF16)

            # normalized global-token output into attnT column 0
            for c in range(CH):
                nc.vector.tensor_tensor(
                    attnT[0:64, c, 0:1], o0_ps[0:64, c, 2 * c:2 * c + 1],
                    r0bc[0:64, 2 * c:2 * c + 1], op=mybir.AluOpType.mult,
                )
                nc.vector.tensor_tensor(
                    attnT[64:128, c, 0:1], o0_ps[64:128, c, 2 * c + 1:2 * c + 2],
                    r0bc[64:128, 2 * c + 1:2 * c + 2], op=mybir.AluOpType.mult,
                )

            def win_back(wg2, c, P_sb, mid=None):
                """P transpose + P@V matmuls + attnT drain for one iteration."""
                PT_ps = pbig.tile([128, 8, 128], BF16, tag="big")
                for r in range(2):
                    for s2 in range(4):
                        nc.tensor.transpose(
                            PT_ps[:, 4 * r + s2, :],
                            P_sb[r][:, 2 * s2:2 * s2 + 2, :].rearrange(
                                "p a b -> p (a b)"
                            ),
                            ident[:, :],
                        )
                PT_sb = pp.tile([128, 8, 128], BF16, tag="PT", bufs=2)
                nc.vector.tensor_copy(PT_sb[:, 0:4, :], PT_ps[:, 0:4, :])
                nc.scalar.copy(PT_sb[:, 4:8, :], PT_ps[:, 4:8, :])
                if mid is not None:
                    mid()  # independent PE work covering the PT-copy latency
                O_ps = [None, None]
                for u in range(2):
                    op = (pbig if u == 0 else pr64).tile(
                        [128, 8, WIN], F32, tag=("big" if u == 0 else "r64"))
                    O_ps[u] = op
                    for b1 in range(2):
                        for s2 in range(4):
                            wj = 4 * s2 + 2 * b1 + u
                            w_abs = 16 * wg2 + wj
                            for r in range(2):
                                h = 2 * c + r
                                nc.tensor.matmul(
                                    op[64 * r:64 * r + 64, 2 * s2 + b1, :],
                                    v_sb[64 * u:64 * u + 64, w_abs // 2,
                                         64 * h:64 * h + 64],
                                    PT_sb[64 * u:64 * u + 64, 4 * r + s2,
                                          64 * b1:64 * b1 + 64],
                                    start=True,
                                    stop=True,
                                )
                cb = 1 + 1024 * wg2
                av = attnT[:, c, cb:cb + 1024].rearrange(
                    "p (a b u q) -> p a b u q", a=4, b=2, u=2)
                for u in range(2):
                    src = O_ps[u][:, :, :].rearrange("p (a b) q -> p a b q", a=4)
                    if u == 0:
                        nc.vector.tensor_copy(av[:, :, :, u, :], src)
                    else:
                        nc.scalar.copy(av[:, :, :, u, :], src)

            def outproj(tq):
                r0 = 128 * tq
                rows = min(128, T - r0)
                ps = pbig.tile([128, D], F32, tag="big")
                for c in range(CH):
                    nc.tensor.matmul(
                        ps[:rows, :],
                        attnT[:, c, r0:r0 + rows],
                        wout_sb[:, c, :],
                        start=(c == 0),
                        stop=(c == CH - 1),
                    )
                ob = posb2.tile([128, D], F32, tag="osb")
                if tq % 2 == 0:
                    nc.vector.tensor_scalar_mul(ob[:rows, :], ps[:rows, :],
                                                1.0 / WSCALE)
                else:
                    nc.scalar.activation(
                        ob[:rows, :], ps[:rows, :],
                        mybir.ActivationFunctionType.Identity,
                        bias=0.0, scale=1.0 / WSCALE,
                    )
                nc.sync.dma_start(out=out_d[r0:r0 + rows, :], in_=ob[:rows, :])

            # Two-stage software pipeline at the emission level (the first two
            # fronts were emitted during the phase-A tail).  Ready
            # output-projection tiles are trickled two per win_back, armed
            # with a one-win_back delay so they never wait on the attnT
            # drains emitted in the same iteration.
            its = [(wg2, c) for wg2 in range(WG2) for c in range(CH)]
            pending = [(its[i], warm[i]) for i in range(len(warm))]
            armed = []
            fresh = []
            next_ready = 0

            def note_ready(bit):
                nonlocal next_ready
                if bit[1] == CH - 1:  # last chunk of a supergroup
                    hi = 8 * (bit[0] + 1) if bit[0] + 1 < WG2 else TQ
                    fresh.extend(range(next_ready, hi))
                    next_ready = hi

            def trickle2():
                for _ in range(2):
                    if armed:
                        outproj(armed.pop(0))

            def step_back():
                (bit, bP) = pending.pop(0)
                win_back(bit[0], bit[1], bP, mid=trickle2)
                note_ready(bit)
                armed.extend(fresh)
                fresh.clear()

            # the warm-up overfilled the pipeline to depth 3 (hiding the
            # phase-A tail chain); drain back to the steady depth of 2 that
            # the PSUM pool rotation is sized for
            step_back()
            for it in its[len(warm):]:
                step_back()
                pending.append((it, win_front(*it)))
            while pending:
                step_back()
            while armed:
                outproj(armed.pop(0))


def build(T=T_FULL):
    nc = bacc.Bacc("TRN2", target_bir_lowering=False, debug=False,
                   num_devices=N_CORES)
    x_d = nc.dram_tensor("x", [T, D], F32, kind="ExternalInput")
    wqkv_d = nc.dram_tensor("w_qkv", [D, 3 * D], F32, kind="ExternalInput")
    wout_d = nc.dram_tensor("w_out", [D, D], F32, kind="ExternalInput")
    out_d = nc.dram_tensor("out", [T, D], F32, kind="ExternalOutput")
    with tile.TileContext(nc) as tc:
        _emit(nc, tc, x_d.ap(), wqkv_d.ap(), wout_d.ap(), out_d.ap(), T)
    nc.compile()
    return nc


_NC_CACHE = {}


def kernel(x, w_qkv, w_out):
    x = np.ascontiguousarray(np.asarray(x, dtype=np.float32))
    w_qkv = np.ascontiguousarray(np.asarray(w_qkv, dtype=np.float32))
    w_out = np.ascontiguousarray(np.asarray(w_out, dtype=np.float32))
    assert x.shape == (B, T_FULL, D)

    if "nc" not in _NC_CACHE:
        _NC_CACHE["nc"] = build(T_FULL)
    nc = _NC_CACHE["nc"]

    in_maps = [
        {"x": x[b], "w_qkv": w_qkv, "w_out": w_out} for b in range(N_CORES)
    ]
    last_err = None
    for _attempt in range(4):
        try:
            res = run_bass_kernel_spmd(nc, in_maps, core_ids=list(range(N_CORES)))
            break
        except Exception as e:  # transient NRT device errors
            last_err = e
            try:  # force a fresh PJRT client before retrying
                import jax
                jax.clear_caches()
                jax.extend.backend.clear_backends()
            except Exception:
                pass
            import time as _time
            _time.sleep(5)
    else:
        raise last_err
    return np.stack([res.results[b]["out"] for b in range(N_CORES)], axis=0)
